# revision 22
# baseline (speedup 1.0000x reference)
"""AWQ linear (int4 group-quantized) matmul on 8 Trainium2 NeuronCores.

out[m, n] = sum_k x[m, k] * W[n, k] + bias[n]
W[n, k] = (q4[n, k] - qzeros[n, k//128]) * qscales[n, k//128]

Column-parallel: shard N=11008 across 8 cores (1376 each), replicate x.
Per core:
  - host repacks qweight nibbles to a k-major bf16 tensor [K, Ns] (small
    ints, exact in bf16) and swizzles x^T (bf16) so every (k-group,
    m-tile) slab is one contiguous DMA
  - device dequantizes W^T[k, n] = (q4 - z)*s into resident SBUF bf16:
    scale/zero rows are DMA-broadcast across partitions (bf16), DVE does
    the two-tensor affine entirely in bf16 (fast path); dequant DMAs are
    issued from the otherwise-idle ACT sequencer so they can never stall
    the Sync queue that feeds x/out traffic
  - matmuls accumulate over k in asymmetric splits (8/8/16 k-tiles) so
    the PE starts ~35us in while later groups still dequantize; partial
    sums accumulate in SBUF via DVE adds, bias fused into split 0;
    dequant DVE work for the next split is interleaved between evictions
"""

import os

import numpy as np
import ml_dtypes

M, K, NFULL = 4096, 4096, 11008
NCORES = 8
NS = NFULL // NCORES          # 1376 out-features per core
P = 128                       # partitions; also the quant group size
MM_FREE = 512                 # psum bank limit (fp32)
XG = 8                        # k-tiles per x-slab group in the host layout

LAST_RESULTS = None           # BassKernelResults of the last kernel() call


def build_nc(k=K, m=M, ns=NS, n_cores=NCORES, splits=(8, 8, 16), mt_block=6, xg=XG):
    """Build + compile the per-core Bass program (SPMD: same NEFF on all cores).

    Block 0 accumulates over k in `splits` (so the PE can start while later
    k-groups still dequantize); the remaining blocks run one full-k span.
    """
    import concourse.bass as bass
    import concourse.mybir as mybir
    import concourse.tile as tile
    from concourse import bacc

    kt_n = k // P
    mt_n = m // P
    assert sum(splits) == kt_n and mt_block <= mt_n
    assert all(s % xg == 0 for s in splits) and all(s % 2 == 0 for s in splits)
    chunks = [(i, min(MM_FREE, ns - i)) for i in range(0, ns, MM_FREE)]

    f32 = mybir.dt.float32
    bf16 = mybir.dt.bfloat16
    ADD = mybir.AluOpType.add
    SUB = mybir.AluOpType.subtract
    MUL = mybir.AluOpType.mult

    nc = bacc.Bacc("TRN2", num_devices=n_cores)
    # xt rows are (kg, mt, p): each (kg, mt) slab is contiguous [128, xg*128]
    xt = nc.dram_tensor("xt", [(kt_n // xg) * mt_n * P, xg * P], bf16, kind="ExternalInput")
    # q4 rows are partitions: q4[p, kt*ns + n] = q4_kmajor[kt*128 + p, n]
    q4 = nc.dram_tensor("q4", [P, kt_n * ns], bf16, kind="ExternalInput")
    # scale/zero rows interleaved: row 2g = scales[g], row 2g+1 = zeros[g]
    szt = nc.dram_tensor("sz", [2 * kt_n, ns], bf16, kind="ExternalInput")
    bias = nc.dram_tensor("bias", [1, ns], f32, kind="ExternalInput")
    out = nc.dram_tensor("out", [m, ns], f32, kind="ExternalOutput")

    with tile.TileContext(nc) as tc:
        with (
            tc.tile_pool(name="persist", bufs=1) as persist,
            tc.tile_pool(name="dq", bufs=2) as dq,
            tc.tile_pool(name="dqt", bufs=1) as dqt,
            tc.tile_pool(name="xp1", bufs=2) as xp1,
            tc.tile_pool(name="xp2", bufs=4) as xp2,
            tc.tile_pool(name="op", bufs=mt_block + 1) as op,
            tc.tile_pool(name="ps", bufs=6, space="PSUM") as ps,
        ):
            w_all = persist.tile([P, kt_n, ns], bf16)
            bias_exp = persist.tile([P, ns], f32)

            def dequant_pair(i):
                """Dequantize k-tiles 2i and 2i+1. DMA descriptor batching:
                one SWDGE load covers both q4 tiles (contiguous per
                partition), one ACT broadcast covers all four scale/zero
                rows -- broadcasts are descriptor-rate-limited, so bytes
                per descriptor is what matters."""
                kt = 2 * i
                q4sl = dq.tile([P, 2, ns], bf16, tag="q4sl")
                nc.gpsimd.dma_start(
                    q4sl[:], q4.ap()[:, kt * ns:(kt + 2) * ns].rearrange(
                        "p (j n) -> p j n", j=2
                    )
                )
                szx = dq.tile([P, 4, ns], bf16, tag="szx")
                src = szt.ap()[2 * kt:2 * kt + 4, :]
                src = bass.AP(src.tensor, src.offset, [[0, P]] + list(src.ap))
                # broadcasts write 1.4MB each: alternate between the two HWDGE
                # queues (ACT, Sync) so delivery is not single-queue-limited
                eng = nc.scalar if i % 2 == 0 else nc.sync
                eng.dma_start(szx[:], src)
                for j in range(2):
                    tmp = dqt.tile([P, ns], bf16, tag="dqtmp")
                    nc.vector.tensor_tensor(
                        tmp[:], q4sl[:, j, :], szx[:, 2 * j + 1, :], SUB
                    )
                    nc.vector.tensor_tensor(
                        w_all[:, kt + j, :], tmp[:], szx[:, 2 * j, :], MUL
                    )

            def x_slab(g0, ng, mt):
                """Load x k-groups g0..g0+ng-1 for m-tile mt: [128, ng, xg*128]."""
                pool = xp1 if ng == 1 else xp2
                xbf = pool.tile([P, ng, xg * P], bf16, tag=f"xbf{ng}")
                for gi in range(ng):
                    r0 = ((g0 + gi) * mt_n + mt) * P
                    nc.sync.dma_start(xbf[:, gi, :], xt.ap()[r0:r0 + P, :])
                return xbf

            def mm_sweep(pst, sz_args, kt0, n_kt, slabs, slab_kts):
                """Accumulate kt0..kt0+n_kt-1 into pst from the given x slabs."""
                nstart, sz = sz_args
                for kl in range(n_kt):
                    kt = kt0 + kl
                    sb_i = next(i for i, (a, b) in enumerate(slab_kts) if a <= kt < b)
                    loc = kt - slab_kts[sb_i][0]
                    nc.tensor.matmul(
                        pst[:, :sz],
                        slabs[sb_i][:, loc // xg, (loc % xg) * P:(loc % xg + 1) * P],
                        w_all[:, kt, nstart:nstart + sz],
                        start=(kl == 0),
                        stop=(kl == n_kt - 1),
                    )

            for i in range(splits[0] // 2):
                dequant_pair(i)
            nc.scalar.dma_start(bias_exp[:], bias.ap().to_broadcast((P, ns)))

            s_n = len(splits)
            s_start = [sum(splits[:i]) for i in range(s_n)]

            # ---- block 0: k-split sweeps, dequant interleaved ----
            outsb = {}
            for si in range(s_n):
                pending = (
                    list(range(s_start[si + 1] // 2,
                               (s_start[si + 1] + splits[si + 1]) // 2))
                    if si + 1 < s_n
                    else []
                )
                per_mi = (len(pending) + mt_block - 1) // mt_block if pending else 0
                for mi in range(mt_block):
                    mt = mi
                    xbf = x_slab(s_start[si] // xg, splits[si] // xg, mt)
                    span = (s_start[si], s_start[si] + splits[si])
                    if si == 0:
                        outsb[mi] = op.tile(
                            [P, ns], f32, tag="outsb", name=f"outsb_0_{mi}"
                        )
                    for nstart, sz in chunks:
                        pst = ps.tile([P, MM_FREE], f32, tag="psum")
                        mm_sweep(pst, (nstart, sz), span[0], splits[si], [xbf], [span])
                        osl = outsb[mi][:, nstart:nstart + sz]
                        if si == 0:
                            nc.vector.tensor_tensor(
                                osl, pst[:, :sz], bias_exp[:, nstart:nstart + sz], ADD
                            )
                        else:
                            nc.vector.tensor_tensor(osl, osl, pst[:, :sz], ADD)
                    for i in pending[mi * per_mi:(mi + 1) * per_mi]:
                        dequant_pair(i)
                    if si == s_n - 1:
                        nc.sync.dma_start(
                            out.ap()[mt * P:(mt + 1) * P, :], outsb[mi][:]
                        )

            # ---- blocks 1+: full-k accumulation spans ----
            half = kt_n // 2
            for mt in range(mt_block, mt_n):
                slabs = [x_slab(0, half // xg, mt), x_slab(half // xg, half // xg, mt)]
                slab_kts = [(0, half), (half, kt_n)]
                osb = op.tile([P, ns], f32, tag="outsb", name=f"outsb_{mt}")
                for nstart, sz in chunks:
                    pst = ps.tile([P, MM_FREE], f32, tag="psum")
                    mm_sweep(pst, (nstart, sz), 0, kt_n, slabs, slab_kts)
                    nc.vector.tensor_tensor(
                        osb[:, nstart:nstart + sz],
                        pst[:, :sz],
                        bias_exp[:, nstart:nstart + sz],
                        ADD,
                    )
                nc.sync.dma_start(out.ap()[mt * P:(mt + 1) * P, :], osb[:])

    nc.compile()
    return nc


def prep_x(x, xg=XG):
    """bf16 x^T swizzled so each (kg, mt) slab is one contiguous [128, xg*128]
    row-block: xt[(kg*mt_n + mt)*128 + p, kl*128 + j] = x[mt*128 + j, (kg*xg + kl)*128 + p]
    """
    m, k = x.shape
    kt_n, mt_n = k // P, m // P
    kg_n = kt_n // xg
    xbf = x.astype(ml_dtypes.bfloat16)
    # [mt, j, kg, kl, p] -> [kg, mt, p, kl, j]
    xs = xbf.reshape(mt_n, P, kg_n, xg, P).transpose(2, 0, 4, 3, 1)
    return np.ascontiguousarray(xs.reshape(kg_n * mt_n * P, xg * P))


def prep_inputs(x, qweight, qscales, qzeros, bias):
    """Host-side shard/layout prep. Returns per-core input maps."""
    x = np.asarray(x)
    qweight = np.asarray(qweight)
    qscales = np.asarray(qscales)
    qzeros = np.asarray(qzeros)
    bias = np.asarray(bias)

    xprep = prep_x(x)

    # Unpack int4 nibbles into k-major bf16 [K, N] (ints 0..15: exact):
    # even k -> low nibble, odd k -> high nibble of byte qweight[n, k//2]
    b = qweight.astype(np.uint8)              # [N, K//2]
    q4 = np.empty((K, NFULL), ml_dtypes.bfloat16)
    q4[0::2, :] = (b & 15).T
    q4[1::2, :] = (b >> 4).T
    kt_n = K // P
    # partition-major: q4p[p, kt, n] = q4[kt*128 + p, n]
    q4p = np.ascontiguousarray(q4.reshape(kt_n, P, NFULL).transpose(1, 0, 2))

    sT = qscales.astype(ml_dtypes.bfloat16).T   # [G, N]
    zT = qzeros.astype(ml_dtypes.bfloat16).T    # [G, N]
    sz = np.empty((2 * kt_n, NFULL), ml_dtypes.bfloat16)
    sz[0::2, :] = sT
    sz[1::2, :] = zT
    bias2d = bias.astype(np.float32).reshape(1, NFULL)

    in_maps = []
    for c in range(NCORES):
        sl = slice(c * NS, (c + 1) * NS)
        in_maps.append(
            {
                "xt": xprep,
                "q4": np.ascontiguousarray(q4p[:, :, sl]).reshape(P, kt_n * NS),
                "sz": np.ascontiguousarray(sz[:, sl]),
                "bias": np.ascontiguousarray(bias2d[:, sl]),
            }
        )
    return in_maps


def kernel(x, qweight, qscales, qzeros, bias):
    global LAST_RESULTS
    from concourse.bass_utils import run_bass_kernel_spmd

    nc = build_nc()
    in_maps = prep_inputs(x, qweight, qscales, qzeros, bias)
    trace = bool(os.environ.get("BASS_AWQ_TRACE"))
    res = run_bass_kernel_spmd(
        nc,
        in_maps,
        core_ids=list(range(NCORES)),
        trace=trace,
        trace_cores=list(range(NCORES)) if trace else None,
    )
    LAST_RESULTS = res
    return np.concatenate([res.results[c]["out"] for c in range(NCORES)], axis=1)
